# revision 4
# baseline (speedup 1.0000x reference)
"""CrossTypeHGNN Trainium2 kernel (v7: packed dual-layer stationaries).

Reference computation (per node type i in {0,1,2}, N=6144, F=64):
    u_i = sum_{j != i} H_ij @ x_j              # layer-1 cross-type aggregation
    h_i = u_i @ W1_i.T + b1_i
    v_i = sum_{j != i} H_ij @ h_j              # layer-2 on hidden features
    out_i = v_i @ W2_i.T + b2_i

Measured regime on these cores: DMA streams ~30MB/core in well under the PE
time, so the kernel is PE-instruction + gather-latency bound.  v6 therefore
minimizes PE matmul instructions and hides collective latency:

  - Matrix m=(i,j) needs x_j for layer 1 and h'_j for layer 2 -- the SAME
    source type.  For matrices whose gathered h'_j arrives before they
    stream, the stationary packs [x_j | h'_j] (64+64 = 128 PE columns) and
    ONE DoubleRow matmul per (t2, psum-half) computes BOTH layers: psum
    partitions 0:64 accumulate layer 1, 64:128 accumulate layer 2.
  - Rowsums (for the exact layer-2 bias term) are computed on the host from
    the SAME fp8 H the device multiplies, freeing the 3 ones-columns that
    previously forced a 67-wide stationary.
  - Stream order [H01 H02 | H12 | H10 | H20 H21]: H12 is interposed between
    the gather-0 kick and its first consumer H10, hiding the collective
    latency behind independent layer-1 work.  H10/H20/H21 run packed.
    H01's layer 2 (needs h1) runs in the PE shadow of the H21 stream from
    the SBUF cache; H02/H12's layer 2 (need h2, the last gather) runs in
    the tail.  Every H element is read from HBM once.
  - Bulk H rides SP/ACT HWDGE rings as PAIRS of 256-row slices (one DMA,
    one completion sem per pair).  Gathered-h stationary halves are written
    straight into the packed stationary tiles by per-rank DMAs on the same
    rings, emitted where the ring's pending work is not yet urgent so the
    wait on the gather sem at the ring head is harmless.  Tiny linears run
    in bf16 (4x fewer PE cycles than fp32).
  - Outputs leave transposed bf16 [3, 64, 768]/core; the host
    upcasts/transposes/concats.
"""

import numpy as np
import ml_dtypes
from contextlib import ExitStack

import concourse.bacc as bacc
import concourse.mybir as mybir
import concourse.tile as tile
from concourse.bass_utils import run_bass_kernel_spmd
from concourse.masks import make_identity

N = 6144
F = 64
CORES = 8
R = N // CORES            # 768 rows per core
T2 = N // 256             # 24 double-contraction tiles (256 rows each)
LT = R // 128             # 6 local 128-row blocks
NH = 384                  # psum half of the 768-wide free dim (one bank)
NQ = T2 // 2              # 12 pair-loads per matrix

PAIRS = [(0, 1), (0, 2), (1, 0), (1, 2), (2, 0), (2, 1)]  # m -> (i, j)
CACHED = (0, 1, 3)        # H01, H02, H12: layer-2 runs later, from SBUF cache
ORDER = (0, 1, 3, 2, 4, 5)

BF16 = mybir.dt.bfloat16
F8 = mybir.dt.float8e4
F32 = mybir.dt.float32
DR = mybir.MatmulPerfMode.DoubleRow

FAKE_GATHER = False       # diagnostic: replace collectives with local copies
UPPER_READOUT = "shift"   # "shift": partition-shift DVE copy; "mm": base-64 moving


def build_module(n_repeats=1, serialize=False):
    """serialize=True inserts an all-engine barrier between repeats so the
    marginal per-repeat time of the repeated NEFF measures the single-shot
    execution time (used by the timing harness)."""
    nc = bacc.Bacc("TRN2", target_bir_lowering=False, debug=False,
                   num_devices=CORES)

    ht8_d = nc.dram_tensor("ht8", [6, T2, 128, 2, R], F8, kind="ExternalInput")
    cs0_d = nc.dram_tensor("cs0", [128, 3, T2, 2, 2, F], F8,
                           kind="ExternalInput")
    s3_d = nc.dram_tensor("s3", [3, 3, R], BF16, kind="ExternalInput")
    w1t_d = nc.dram_tensor("w1t", [F, 3, F], BF16, kind="ExternalInput")
    w2e_d = nc.dram_tensor("w2e", [67, 3, F], BF16, kind="ExternalInput")
    b2_d = nc.dram_tensor("b2", [F, 3, 1], F32, kind="ExternalInput")
    outT_d = nc.dram_tensor("outT", [3, F, R], BF16, kind="ExternalOutput")

    with tile.TileContext(nc) as tc, ExitStack() as ctx:
        const = ctx.enter_context(tc.tile_pool(name="const", bufs=1))
        cspool = ctx.enter_context(tc.tile_pool(name="cspool", bufs=2))
        cache = ctx.enter_context(tc.tile_pool(name="cache", bufs=1))
        hstream = ctx.enter_context(tc.tile_pool(name="hstream", bufs=10))
        work = ctx.enter_context(tc.tile_pool(name="work", bufs=2))
        pacc = ctx.enter_context(tc.tile_pool(name="pacc", bufs=6, space="PSUM"))
        pmisc = ctx.enter_context(tc.tile_pool(name="pmisc", bufs=2, space="PSUM"))
        dram = ctx.enter_context(tc.tile_pool(name="dram", bufs=1, space="DRAM"))

        # ---- persistent constants ------------------------------------------
        w1_sb = const.tile([F, 3, F], BF16)
        nc.gpsimd.dma_start(w1_sb[:], w1t_d[:])
        w2_sb = const.tile([67, 3, F], BF16)
        nc.gpsimd.dma_start(w2_sb[:], w2e_d[:])
        b2_sb = const.tile([F, 3, 1], F32)
        nc.gpsimd.dma_start(b2_sb[:], b2_d[:])
        s_sb = const.tile([67, 3, R], BF16)
        nc.gpsimd.dma_start(s_sb[F:67, :, :], s3_d[:])
        identity = const.tile([128, 128], BF16)
        make_identity(nc, identity)

        # cross-layer cache for the 3 late-consumed matrices (pair tiles)
        cache_tiles = {
            (m, q): cache.tile([128, 2, 2, R], F8, name=f"hc_{m}_{q}",
                               tag=f"hc_{m}_{q}")
            for m in CACHED for q in range(NQ)
        }

        def cached_slice(m, t2):
            return cache_tiles[(m, t2 >> 1)][:, t2 & 1]

        for rep in range(n_repeats):
            if serialize and rep > 0:
                tc.strict_bb_all_engine_barrier()

            # packed stationaries: [p, t2, half, k, f], half 0 = x_j (host),
            # half 1 = h'_j (device, written after the type-j gather)
            cs = [
                cspool.tile([128, T2, 2, 2, F], F8, name=f"cs_{rep}_{j}",
                            tag=f"cs_{j}")
                for j in range(3)
            ]
            nc.sync.dma_start(cs[1][:], cs0_d[:, 1])
            nc.scalar.dma_start(cs[2][:], cs0_d[:, 2])
            nc.scalar.dma_start(cs[0][:], cs0_d[:, 0])

            ag_in, ag_out = {}, {}
            for j in range(3):
                ag_in[j] = dram.tile([128, LT // 2, 2, F], F8,
                                     name=f"agi_{rep}_{j}", tag=f"agi{rep}_{j}")
                ag_out[j] = dram.tile(
                    [CORES, 128, LT // 2, 2, F], F8,
                    addr_space="Local" if FAKE_GATHER else "Shared",
                    name=f"ago_{rep}_{j}", tag=f"ago{rep}_{j}",
                )

            acc0 = [pacc.tile([F, NH], F32, name=f"a0_{rep}_{hh}", tag="acc")
                    for hh in (0, 1)]
            p1 = [pacc.tile([128, NH], F32, name=f"p1_{rep}_{hh}", tag="acc")
                  for hh in (0, 1)]
            p2 = [pacc.tile([128, NH], F32, name=f"p2_{rep}_{hh}", tag="acc")
                  for hh in (0, 1)]
            # H10 continues the v1 region (p1 upper) without a start flag,
            # so pre-zero it
            for hh in (0, 1):
                nc.vector.memset(p1[hh][F:2 * F, :], 0.0)

            def load_pair(m, q):
                """one DMA for t2 slices (2q, 2q+1); returns the pair tile."""
                eng = nc.sync if q & 1 == 0 else nc.scalar
                if m in CACHED:
                    mt = cache_tiles[(m, q)]
                else:
                    mt = hstream.tile([128, 2, 2, R], F8, name="hs", tag="hs")
                eng.dma_start(
                    mt[:], ht8_d[m, 2 * q:2 * q + 2].rearrange(
                        "t p k r -> p t k r")
                )
                return mt

            def finish_type(i):
                """type-i aggregate done: bf16 linear, transpose, gather, and
                write the gathered h' into the packed stationaries."""
                src = acc0 if i == 0 else (p1 if i == 1 else p2)
                u1 = work.tile([F, R], BF16, name=f"u1_{rep}_{i}", tag="u")
                hT = work.tile([F, R], BF16, name=f"hT_{rep}_{i}", tag="t16")
                for hh in (0, 1):
                    nc.vector.tensor_copy(
                        u1[:, hh * NH:(hh + 1) * NH], src[hh][0:F, :]
                    )
                    lp = pmisc.tile([F, NH], F32, name=f"lp1_{rep}_{i}_{hh}",
                                    tag="misc")
                    nc.tensor.matmul(
                        lp[:], w1_sb[:, i, :], u1[:, hh * NH:(hh + 1) * NH],
                        start=True, stop=True,
                    )
                    nc.vector.tensor_copy(hT[:, hh * NH:(hh + 1) * NH], lp[:])
                hnat = work.tile([128, LT // 2, 2, F], F8,
                                 name=f"hnat_{rep}_{i}", tag="hnat")
                for lt in range(LT):
                    tp = pmisc.tile([128, F], BF16, name=f"tp_{rep}_{i}_{lt}",
                                    tag="misc")
                    nc.tensor.transpose(
                        tp[:], hT[:, lt * 128:(lt + 1) * 128],
                        identity[0:F, 0:F],
                    )
                    nc.vector.tensor_copy(hnat[:, lt >> 1, lt & 1, :], tp[:])
                nc.gpsimd.dma_start(ag_in[i][:], hnat[:])
                if FAKE_GATHER:
                    for rank in range(CORES):
                        nc.gpsimd.dma_start(ag_out[i][rank], ag_in[i][:])
                else:
                    nc.gpsimd.collective_compute(
                        "AllGather",
                        mybir.AluOpType.bypass,
                        replica_groups=[list(range(CORES))],
                        ins=[ag_in[i][:]],
                        outs=[ag_out[i][:]],
                    )
                # the gathered-h stationary loads are NOT issued here: they
                # ride the bulk HWDGE rings, emitted by the stream loop at a
                # point where the rings' pending work is not yet needed (so
                # the wait on the gather sem at the ring head is harmless)

            def upper_copy(u2dst, ptile, hh):
                if UPPER_READOUT == "shift":
                    nc.vector.tensor_copy(
                        u2dst[0:F, hh * NH:(hh + 1) * NH],
                        ptile[hh][F:2 * F, :],
                    )
                else:
                    raise NotImplementedError

            def linear2(i, ptile=None, vlow=None, extra=None):
                u2 = work.tile([67, R], BF16, name=f"u2_{rep}_{i}", tag="u")
                for hh in (0, 1):
                    if ptile is not None:
                        upper_copy(u2, ptile, hh)
                    elif extra is not None:
                        nc.vector.tensor_add(
                            u2[0:F, hh * NH:(hh + 1) * NH],
                            vlow[hh][:],
                            extra[:, hh * NH:(hh + 1) * NH],
                        )
                    else:
                        nc.vector.tensor_copy(
                            u2[0:F, hh * NH:(hh + 1) * NH], vlow[hh][:]
                        )
                nc.vector.tensor_copy(u2[F:67, :], s_sb[F:67, i, :])
                od = work.tile([F, R], BF16, name=f"od_{rep}_{i}", tag="t16")
                for hh in (0, 1):
                    lp = pmisc.tile([F, NH], F32, name=f"lp2_{rep}_{i}_{hh}",
                                    tag="misc")
                    nc.tensor.matmul(
                        lp[:], w2_sb[0:67, i, :], u2[:, hh * NH:(hh + 1) * NH],
                        start=True, stop=True,
                    )
                    nc.vector.tensor_scalar_add(
                        od[:, hh * NH:(hh + 1) * NH], lp[:], b2_sb[:, i, :]
                    )
                (nc.sync if i != 1 else nc.scalar).dma_start(outT_d[i], od[:])

            # ---- the fused stream ------------------------------------------
            # x-only / h-only / packed stationary slices of cs[j]
            def stat_x(j, t2):
                return cs[j][:, t2, 0]

            def stat_h(j, t2):
                return cs[j][:, t2, 1]

            def stat_xh(j, t2):
                # free dims must be (k, cols) with cols = half*64+f
                return cs[j][:, t2].rearrange("p h k f -> p k h f")

            def emit_stt_loads(ph):
                """write gathered h' of phase ph into the packed stationary
                h-halves: per-rank 3-dim DMAs, 4 on each bulk ring."""
                for rank in range(CORES):
                    eng = nc.sync if rank & 1 == 0 else nc.scalar
                    eng.dma_start(
                        cs[ph][:, 3 * rank:3 * rank + 3, 1],
                        ag_out[ph][rank],
                    )

            # stt loads are emitted into the ring queues just before the
            # listed matrix's bulk loads (late enough that the wait on the
            # gather stalls nothing urgent, early enough for the consumer)
            STT_BEFORE = {2: 0, 5: 1}   # m -> phase

            shadow = None
            shadow_step = 0
            for m in ORDER:
                i, j = PAIRS[m]
                if m in STT_BEFORE:
                    emit_stt_loads(STT_BEFORE[m])
                if m == 5:
                    shadow = [
                        pmisc.tile([F, NH], F32, name=f"sh_{rep}_{hh}",
                                   tag="misc")
                        for hh in (0, 1)
                    ]
                for q in range(NQ):
                    pair = load_pair(m, q)
                    for tt in (0, 1):
                        t2 = 2 * q + tt
                        mv = pair[:, tt]
                        if m == 0 or m == 1:    # L1 only -> acc0
                            for hh in (0, 1):
                                nc.tensor.matmul(
                                    acc0[hh][:], stat_x(j, t2),
                                    mv[:, :, hh * NH:(hh + 1) * NH],
                                    start=(m == 0 and t2 == 0),
                                    stop=(m == 1 and t2 == T2 - 1),
                                    perf_mode=DR,
                                )
                        elif m == 3:            # L1 only -> p1 lower
                            for hh in (0, 1):
                                nc.tensor.matmul(
                                    p1[hh][0:F, :], stat_x(j, t2),
                                    mv[:, :, hh * NH:(hh + 1) * NH],
                                    start=t2 == 0, stop=False,
                                    perf_mode=DR, skip_group_check=True,
                                )
                        elif m == 2:            # packed -> p1 (both layers)
                            for hh in (0, 1):
                                nc.tensor.matmul(
                                    p1[hh][:], stat_xh(j, t2),
                                    mv[:, :, hh * NH:(hh + 1) * NH],
                                    start=False, stop=False,
                                    perf_mode=DR, skip_group_check=True,
                                )
                        elif m == 4:            # packed -> p2
                            for hh in (0, 1):
                                nc.tensor.matmul(
                                    p2[hh][:], stat_xh(j, t2),
                                    mv[:, :, hh * NH:(hh + 1) * NH],
                                    start=t2 == 0, stop=False,
                                    perf_mode=DR, skip_group_check=True,
                                )
                        else:                   # m == 5: packed + shadow
                            for hh in (0, 1):
                                nc.tensor.matmul(
                                    p2[hh][:], stat_xh(j, t2),
                                    mv[:, :, hh * NH:(hh + 1) * NH],
                                    start=False, stop=t2 == T2 - 1,
                                    perf_mode=DR, skip_group_check=True,
                                )
                            s_t2 = shadow_step
                            for s_hh in (0, 1):
                                nc.tensor.matmul(
                                    shadow[s_hh][:], stat_h(1, s_t2),
                                    cached_slice(0, s_t2)[
                                        :, :, s_hh * NH:(s_hh + 1) * NH],
                                    start=s_t2 == 0, stop=s_t2 == T2 - 1,
                                    perf_mode=DR, skip_group_check=True,
                                )
                            shadow_step += 1
                if m == 1:
                    finish_type(0)
                elif m == 2:
                    finish_type(1)
                elif m == 5:
                    pass

            # spill the shadow chain so pmisc frees for linear1(2)
            vp0 = work.tile([F, R], BF16, name=f"vp0_{rep}", tag="vp0")
            for hh in (0, 1):
                nc.vector.tensor_copy(vp0[:, hh * NH:(hh + 1) * NH],
                                      shadow[hh][:])
            finish_type(2)
            emit_stt_loads(2)   # rings are idle now; they just wait gather-2

            # v2 = H20@h0 + H21@h1 complete -> out2 leaves during the tail
            linear2(2, ptile=p2)

            # spill v1's streamed half (H10@h0, p1 upper) to SBUF: a DR
            # matmul cannot target psum partition base 64 (walrus ISA check
            # rejects DoubleRow + tile_position), so the tail H12 chain gets
            # its own base-0 psum and the halves are summed on readout
            vv1 = work.tile([F, R], BF16, name=f"vv1_{rep}", tag="vp0")
            for hh in (0, 1):
                nc.vector.tensor_copy(vv1[:, hh * NH:(hh + 1) * NH],
                                      p1[hh][F:2 * F, :])

            # ---- tail: the two h2-consumers, from cache --------------------
            vt = [pacc.tile([F, NH], F32, name=f"vt_{rep}_{hh}", tag="acc")
                  for hh in (0, 1)]
            vt2 = [pacc.tile([F, NH], F32, name=f"vt2_{rep}_{hh}", tag="acc")
                   for hh in (0, 1)]
            for t2 in range(T2):
                for hh in (0, 1):
                    nc.tensor.matmul(
                        vt[hh][:], stat_h(2, t2),
                        cached_slice(1, t2)[:, :, hh * NH:(hh + 1) * NH],
                        start=t2 == 0, stop=t2 == T2 - 1, perf_mode=DR,
                        skip_group_check=True,
                    )
                for hh in (0, 1):
                    nc.tensor.matmul(
                        vt2[hh][:], stat_h(2, t2),
                        cached_slice(3, t2)[:, :, hh * NH:(hh + 1) * NH],
                        start=t2 == 0, stop=t2 == T2 - 1, perf_mode=DR,
                        skip_group_check=True,
                    )
            linear2(0, vlow=vt, extra=vp0)
            linear2(1, vlow=vt2, extra=vv1)

    nc.compile()
    return nc


def prep_inputs(inputs):
    """Host-side shard/transpose/cast. Returns per-core input maps."""
    fp8 = ml_dtypes.float8_e4m3
    bf16 = ml_dtypes.bfloat16

    # ht8[core, m, t2, p, k, r] = N * H_m[768*core + r, 256*t2 + 128*k + p]
    ht8_all = np.empty((CORES, 6, T2, 128, 2, R), dtype=fp8)
    for m, (i, j) in enumerate(PAIRS):
        Hm = np.asarray(inputs[f"H{i}{j}"], dtype=np.float32)
        scaled = Hm * np.float32(N)
        perm = scaled.reshape(CORES, R, T2, 2, 128).transpose(0, 2, 4, 3, 1)
        ht8_all[:, m] = perm.astype(fp8)

    # packed stationary x-halves: cs0[p, j, t2, 0, k, f] = x_j[256t2+128k+p, f]
    cs0 = np.zeros((128, 3, T2, 2, 2, F), dtype=fp8)
    for j in range(3):
        xj = np.asarray(inputs[f"x{j}"], dtype=np.float32)
        cs0[:, j, :, 0, :, :] = (
            xj.reshape(T2, 2, 128, F).transpose(2, 0, 1, 3).astype(fp8)
        )

    # host rowsums of the SAME fp8 H the device multiplies (exact bias term):
    # s3[core, jrow, i, r] = sum over cols of (N*H_{i,jrow})_fp8 row r
    s3_all = np.zeros((CORES, 3, 3, R), dtype=bf16)
    for c in range(CORES):
        for m, (i, j) in enumerate(PAIRS):
            s3_all[c, j, i, :] = (
                ht8_all[c, m].astype(np.float32).sum(axis=(0, 1, 2))
                .astype(bf16)
            )

    w1t = np.ascontiguousarray(
        np.stack(
            [np.asarray(inputs[f"W1_{i}"], dtype=np.float32).T
             for i in range(3)],
            axis=1,
        )
    ).astype(bf16)  # [fin, 3, fout], no scale: h'_scaled = (N*u) @ W1.T

    w2e = np.zeros((67, 3, F), dtype=np.float32)
    inv_n2 = np.float32(1.0 / (float(N) * float(N)))
    inv_n = np.float32(1.0 / float(N))
    for i in range(3):
        W2 = np.asarray(inputs[f"W2_{i}"], dtype=np.float32)
        w2e[0:F, i, :] = W2.T * inv_n2
        for j in range(3):
            if j == i:
                continue
            b1j = np.asarray(inputs[f"b1_{j}"], dtype=np.float32)
            w2e[F + j, i, :] = (b1j @ W2.T) * inv_n
    w2e = w2e.astype(bf16)

    b2 = np.ascontiguousarray(
        np.stack(
            [np.asarray(inputs[f"b2_{i}"], dtype=np.float32).reshape(F, 1)
             for i in range(3)],
            axis=1,
        )
    )

    shared = {"cs0": cs0, "w1t": w1t, "w2e": w2e, "b2": b2}
    return [
        {"ht8": np.ascontiguousarray(ht8_all[c]), "s3": s3_all[c], **shared}
        for c in range(CORES)
    ]


_CACHED_NC = None


def get_module():
    global _CACHED_NC
    if _CACHED_NC is None:
        _CACHED_NC = build_module()
    return _CACHED_NC


def kernel(**inputs):
    import time

    nc = get_module()
    in_maps = prep_inputs(inputs)
    last_exc = None
    for attempt in range(3):
        try:
            res = run_bass_kernel_spmd(nc, in_maps, core_ids=list(range(CORES)))
            break
        except Exception as exc:  # transient NRT device errors observed on axon
            last_exc = exc
            time.sleep(5.0)
    else:
        raise last_exc
    outs = []
    for i in range(3):
        outs.append(
            np.ascontiguousarray(
                np.concatenate(
                    [res.results[c]["outT"][i].astype(np.float32).T
                     for c in range(CORES)],
                    axis=0,
                )
            )
        )
    return tuple(outs)


if __name__ == "__main__":
    rng = np.random.default_rng(0)
    inputs = {}
    for i in range(3):
        inputs[f"x{i}"] = rng.standard_normal((N, F), dtype=np.float32)
    for i, j in PAIRS:
        inputs[f"H{i}{j}"] = rng.random((N, N), dtype=np.float32) / N
    for i in range(3):
        inputs[f"W1_{i}"] = rng.standard_normal((F, F), dtype=np.float32) * 0.05
        inputs[f"b1_{i}"] = rng.standard_normal((F,), dtype=np.float32) * 0.05
        inputs[f"W2_{i}"] = rng.standard_normal((F, F), dtype=np.float32) * 0.05
        inputs[f"b2_{i}"] = rng.standard_normal((F,), dtype=np.float32) * 0.05

    out = kernel(**inputs)

    def ref(inp):
        u = [None] * 3
        u[0] = inp["H01"] @ inp["x1"] + inp["H02"] @ inp["x2"]
        u[1] = inp["H10"] @ inp["x0"] + inp["H12"] @ inp["x2"]
        u[2] = inp["H20"] @ inp["x0"] + inp["H21"] @ inp["x1"]
        h = [u[i] @ inp[f"W1_{i}"].T + inp[f"b1_{i}"] for i in range(3)]
        v = [None] * 3
        v[0] = inp["H01"] @ h[1] + inp["H02"] @ h[2]
        v[1] = inp["H10"] @ h[0] + inp["H12"] @ h[2]
        v[2] = inp["H20"] @ h[0] + inp["H21"] @ h[1]
        return tuple(v[i] @ inp[f"W2_{i}"].T + inp[f"b2_{i}"] for i in range(3))

    exp = ref(inputs)
    for i in range(3):
        a, e = out[i], exp[i]
        rel = np.abs(a - e).max() / np.abs(e).max()
        print(f"out{i}: absmax-rel err {rel:.3e}")
